# revision 1
# baseline (speedup 1.0000x reference)
"""Trainium2 Bass kernel for nn_CHConv — windowed deformable 3x3 conv, v2.

Design: geometry (offsets/windows/weights) is image-independent (scale and
offset_base carry no batch dim). 8 cores: core c handles image c//4, output
rows {4*j + c%4, j=0..31}. One chunk = one output row (256 px). Per chunk:
  1. DMA a 5-row j-duplicated slab slice [128=(j,c), 5*272] bf16.
  2. DMA compact per-row bilinear weights [16, free] and replicate to 128
     partitions with 3 partition-doubling SBUF DMAs (j-halves).
  3. DVE builds T = slab-window * W per windowed tap (4D APs, no gather).
  4. Small dma_gather for "wrap" taps (equirectangular seam) whose live
     pixels form contiguous column intervals; weighted on DVE.
  5. k-major accumulating matmuls into one PSUM bank [128f, 256px]; wrap
     matmuls land on their column interval. ACT copies PSUM out, DMA to HBM.
"""
import numpy as np
from contextlib import ExitStack

import concourse.bass as bass
import concourse.bacc as bacc
import concourse.mybir as mybir
import concourse.tile as tile
from concourse.bass_utils import run_bass_kernel_spmd
from ml_dtypes import bfloat16

H, W, K, C, F, B = 128, 256, 9, 64, 128, 2
NCH = 32          # chunks (rows) per core
AW = 5            # slab rows per chunk
SLABW = 272       # slab cols (2 left pad, 14 right pad)
NPIX = H * W
NCORES = 8

_BF16 = mybir.dt.bfloat16
_F32 = mybir.dt.float32
_I16 = mybir.dt.int16


# ---------------------------------------------------------------- host plan
def geometry(scale, offset_base):
    off = (offset_base.astype(np.float32) * scale.astype(np.float32)).reshape(
        H, W, K, 2)
    ti, tj = np.meshgrid(np.arange(3), np.arange(3), indexing="ij")
    ti = ti.reshape(-1).astype(np.float32)
    tj = tj.reshape(-1).astype(np.float32)
    ys = (np.arange(H, dtype=np.float32)[:, None, None] - 1.0 + ti[None, None]
          + off[..., 0])
    xs = (np.arange(W, dtype=np.float32)[None, :, None] - 1.0 + tj[None, None]
          + off[..., 1])
    y0 = np.floor(ys); x0 = np.floor(xs)
    fy = (ys - y0).astype(np.float32); fx = (xs - x0).astype(np.float32)
    y0 = y0.astype(np.int64); x0 = x0.astype(np.int64)

    def v(yi, xi):
        return (((yi >= 0) & (yi < H) & (xi >= 0) & (xi < W))
                .astype(np.float32))
    w = np.zeros((H, W, K, 2, 2), np.float32)
    w[..., 0, 0] = (1 - fy) * (1 - fx) * v(y0, x0)
    w[..., 0, 1] = (1 - fy) * fx * v(y0, x0 + 1)
    w[..., 1, 0] = fy * (1 - fx) * v(y0 + 1, x0)
    w[..., 1, 1] = fy * fx * v(y0 + 1, x0 + 1)
    return y0, x0, w


def make_plan(y0, x0, w):
    """Returns per-chunk meta: list of dicts with 'win' and 'wrap' tap lists
    plus free-dim layout offsets (slots are k-major)."""
    live = w.sum(axis=(3, 4)) > 0
    dy = y0 - np.arange(H)[:, None, None]
    dx = x0 - np.arange(W)[None, :, None]
    plan = []
    for ci in range(NCH):
        rows = [4 * ci + r for r in range(4)]
        win, wrap = [], []
        for k in range(K):
            lv = np.stack([live[h, :, k] for h in rows])
            if not lv.any():
                continue
            dyr = np.stack([dy[h, :, k] for h in rows])
            dxr = np.stack([dx[h, :, k] for h in rows])
            dmn, dmx = int(dxr[lv].min()), int(dxr[lv].max())
            ymn, ymx = int(dyr[lv].min()), int(dyr[lv].max())
            awid, bwid = ymx - ymn + 2, dmx - dmn + 2
            if bwid <= 5 and awid <= 4 and ymn >= -2 and ymx + 1 <= 2:
                win.append(dict(k=k, amin=ymn + 2, awid=awid, bmin=dmn,
                                bwid=bwid, nslot=awid * (bwid - 1)))
            else:
                ws = np.where(lv.any(axis=0))[0]
                wrap.append(dict(k=k, wlo=int(ws.min()),
                                 width=int(ws.max() - ws.min() + 1)))
        assert win, f"chunk {ci} has no windowed tap"
        soff = 0
        for t in win:
            t["soff"] = soff
            soff += t["nslot"]
        nw = sum(t["width"] for t in wrap)
        nwpad = ((nw + 127) // 128) * 128 if nw else 0
        noff = 0
        for t in wrap:
            t["noff"] = noff
            noff += t["width"]
        plan.append(dict(ci=ci, win=win, wrap=wrap, nslots=soff, nw=nw,
                         nwpad=nwpad, wfree=soff * W + 2 * nwpad))
    return plan


def plan_sig(plan):
    sig = []
    for ch in plan:
        sig.append((ch["nslots"], ch["nwpad"],
                    tuple((t["k"], t["amin"], t["awid"], t["bmin"], t["bwid"])
                          for t in ch["win"]),
                    tuple((t["k"], t["wlo"], t["width"]) for t in ch["wrap"])))
    return tuple(sig)


# ---------------------------------------------------------------- bass build
def build_bass(plan):
    totw = sum(ch["wfree"] for ch in plan)
    totn = sum(ch["nwpad"] for ch in plan)
    nc = bacc.Bacc("TRN2", target_bir_lowering=False, debug=False)
    slab = nc.dram_tensor("slab", [NCH // 2, 128, 2 * AW * SLABW], _BF16,
                          kind="ExternalInput")
    wcomp = nc.dram_tensor("wcomp", [128, totw], _BF16, kind="ExternalInput")
    widx = nc.dram_tensor("widx", [128, max(totn // 16, 16)], _I16,
                          kind="ExternalInput")
    xd = nc.dram_tensor("xd", [NPIX, 4 * C], _BF16, kind="ExternalInput")
    kdup = nc.dram_tensor("kdup", [128, K * F], _BF16, kind="ExternalInput")
    out = nc.dram_tensor("out", [F, NCH * W], _F32, kind="ExternalOutput")

    woffs, ioffs = [], []
    wo = io = 0
    for ch in plan:
        woffs.append(wo); ioffs.append(io)
        wo += ch["wfree"]; io += ch["nwpad"]

    with ExitStack() as ctx:
        tc = ctx.enter_context(tile.TileContext(nc))
        kp = ctx.enter_context(tc.tile_pool(name="kp", bufs=1))
        idxp = ctx.enter_context(tc.tile_pool(name="idxp", bufs=1))
        slp = ctx.enter_context(tc.tile_pool(name="slp", bufs=4))
        wbp = ctx.enter_context(tc.tile_pool(name="wbp", bufs=3))
        twp = ctx.enter_context(tc.tile_pool(name="twp", bufs=2))
        gtp = ctx.enter_context(tc.tile_pool(name="gtp", bufs=3))
        trp = ctx.enter_context(tc.tile_pool(name="trp", bufs=2))
        op_ = ctx.enter_context(tc.tile_pool(name="op", bufs=3))
        psp = ctx.enter_context(tc.tile_pool(name="psp", bufs=4, space="PSUM"))

        kd = kp.tile([128, K * F], _BF16)
        nc.sync.dma_start(out=kd[:], in_=kdup[:, :])
        idx_t = None
        if totn:
            idx_t = idxp.tile([128, totn // 16], _I16)
            nc.sync.dma_start(out=idx_t[:], in_=widx[:, 0:totn // 16])

        sls, wbs, gts = {}, {}, {}
        NP = NCH // 2  # paired chunks

        def stage_load(t):
            chA, chB = plan[2 * t], plan[2 * t + 1]
            sls[t] = slp.tile([128, 2 * AW * SLABW], _BF16, name=f"sl{t}",
                              tag="sl")
            nc.scalar.dma_start(out=sls[t][:], in_=slab[t, :, :])
            wf = chA["wfree"] + chB["wfree"]
            wbs[t] = wbp.tile([128, wf], _BF16, name=f"wb{t}", tag="wb")
            nc.sync.dma_start(out=wbs[t][:],
                               in_=wcomp[:, woffs[2 * t]:woffs[2 * t] + wf])

        def stage_gather(t):
            chA, chB = plan[2 * t], plan[2 * t + 1]
            ng = chA["nwpad"] + chB["nwpad"]
            if ng:
                gts[t] = gtp.tile([128, 2, ng], _BF16, name=f"gt{t}", tag="gt")
                nc.gpsimd.dma_gather(
                    out_ap=gts[t][:], in_ap=xd[:, :],
                    idxs_ap=idx_t[:, ioffs[2 * t] // 16:
                                  (ioffs[2 * t] + ng) // 16],
                    num_idxs=ng, num_idxs_reg=ng,
                    elem_size=4 * C, transpose=True, single_packet=False,
                )

        def stage_compute(t):
            chA, chB = plan[2 * t], plan[2 * t + 1]
            sl, wb = sls.pop(t), wbs.pop(t)
            ng = chA["nwpad"] + chB["nwpad"]
            nslT = chA["nslots"] + chB["nslots"]
            tw = twp.tile([128, nslT, W], _BF16, name=f"tw{t}", tag="tw")
            tr = (trp.tile([128, 2, ng], _BF16, name=f"tr{t}", tag="tr")
                  if ng else None)
            psA = psp.tile([128, W], _F32, space="PSUM",
                           name=f"psA{t}", tag="psA")
            psB = psp.tile([128, W], _F32, space="PSUM",
                           name=f"psB{t}", tag="psB")
            pss = (psA, psB)

            for s, ch in ((0, chA), (1, chB)):
                slo = s * AW * SLABW
                swoff = s * chA["wfree"]
                ssoff = s * chA["nslots"]
                base = sl[:]
                nslots = ch["nslots"]
                for tp in ch["win"]:
                    a0, b0 = tp["amin"], tp["bmin"]
                    aw_, bw_ = tp["awid"], tp["bwid"]
                    s0 = ssoff + tp["soff"]
                    src4 = bass.AP(
                        base.tensor,
                        base.offset + slo + a0 * SLABW + 2 + b0,
                        [[2 * AW * SLABW, 128], [SLABW, aw_], [1, bw_ - 1],
                         [1, W]],
                    )
                    dst = tw[:, s0:s0 + tp["nslot"], :].rearrange(
                        "p (a bs) x -> p a bs x", a=aw_)
                    wsl = wb[:, swoff + tp["soff"] * W:
                             swoff + (tp["soff"] + tp["nslot"]) * W].rearrange(
                        "p (a bs x) -> p a bs x", a=aw_, bs=bw_ - 1)
                    nc.vector.tensor_tensor(out=dst, in0=src4, in1=wsl,
                                            op=mybir.AluOpType.mult)
                if ch["nwpad"]:
                    goff = s * chA["nwpad"]
                    gt = gts[t][:, :, goff:goff + ch["nwpad"]]
                    trs = tr[:, :, goff:goff + ch["nwpad"]]
                    wsl = wb[:, swoff + nslots * W:
                             swoff + nslots * W + 2 * ch["nwpad"]].rearrange(
                        "p (i n) -> p i n", i=2)
                    nc.vector.tensor_tensor(out=trs, in0=gt, in1=wsl,
                                            op=mybir.AluOpType.mult)

            # k-merged MMs across the two sub-rows, each into its OWN
            # PSUM bank (per-bank chains stay self-contained)
            mm_list = []
            for s, ch in ((0, chA), (1, chB)):
                ssoff = s * chA["nslots"]
                goff = s * chA["nwpad"]
                for tp in ch["win"]:
                    for sx in range(tp["nslot"]):
                        mm_list.append((tp["k"], s, "w",
                                        ssoff + tp["soff"] + sx))
                for tp in ch["wrap"]:
                    for i in range(2):
                        mm_list.append((tp["k"], s, "g",
                                        (tp["wlo"], tp["width"],
                                         goff + tp["noff"], i)))
            mm_list.sort(key=lambda m: (m[0], m[1]))
            head = []
            for s in (0, 1):
                for m in mm_list:
                    if m[1] == s and m[2] == "w":
                        head.append(m)
                        mm_list.remove(m)
                        break
            mm_list = head + mm_list
            started = [False, False]
            lasts = {}
            for idx, (k, s, kind, args) in enumerate(mm_list):
                lasts[s] = idx
            for idx, (k, s, kind, args) in enumerate(mm_list):
                stop = (idx == lasts[s])
                if kind == "w":
                    st = not started[s]
                    started[s] = True
                    nc.tensor.matmul(
                        pss[s][:], lhsT=kd[:, k * F:(k + 1) * F],
                        rhs=tw[:, args, :], start=st, stop=stop)
                else:
                    wlo, wd, noff, i = args
                    nc.tensor.matmul(
                        pss[s][:, wlo:wlo + wd],
                        lhsT=kd[:, k * F:(k + 1) * F],
                        rhs=tr[:, i, noff:noff + wd],
                        start=False, stop=stop)
            if ng:
                gts.pop(t)

            ob = op_.tile([128, 2 * W], _F32, name=f"ob{t}", tag="ob")
            nc.vector.tensor_scalar_mul(ob[:, 0:W], psA[:], 1.0)
            nc.scalar.copy(out=ob[:, W:2 * W], in_=psB[:])
            nc.scalar.dma_start(out=out[:, 2 * t * W:2 * (t + 1) * W],
                                in_=ob[:])

        stage_gather(0)
        for t in range(NP + 2):
            if t < NP:
                stage_load(t)
                if t + 1 < NP:
                    stage_gather(t + 1)
            if 0 <= t - 2 < NP:
                stage_compute(t - 2)
    nc.finalize()
    return nc


# ---------------------------------------------------------------- host pack
def pack_inputs(x, kern, plan, y0a, x0a, wts):
    """Build per-core input dicts."""
    xbf = x.astype(np.float32)
    # padded image for slabs (extra col for j=1 shift)
    xp = np.zeros((B, H + AW, SLABW + 1, C), np.float32)
    xp[:, 2:2 + H, 2:2 + W, :] = xbf
    xp = xp.astype(bfloat16)

    # patch table per image (as baseline)
    xpad = np.pad(xbf, [(0, 0), (0, 1), (0, 1), (0, 0)])
    xd_all = np.empty((B, H, W, 2, 2, C), np.float32)
    for i in range(2):
        for j in range(2):
            xd_all[:, :, :, i, j, :] = xpad[:, i:i + H, j:j + W, :]
    xd_all = xd_all.reshape(B, NPIX, 4 * C).astype(bfloat16)

    km = kern.reshape(F, C, K).astype(np.float32)
    kdv = np.transpose(km, (1, 2, 0)).reshape(C, K * F)
    kdup0 = np.concatenate([kdv, kdv], axis=0)
    cc = np.arange(C)
    pperm0 = np.empty((2, C), np.int64)
    for j in range(2):
        pperm0[j] = (cc // 16) * 32 + j * 16 + (cc % 16)
    kdup_p = np.empty_like(kdup0)
    for j in range(2):
        kdup_p[pperm0[j]] = kdup0[j * C:(j + 1) * C]
    kdup_p = kdup_p.astype(bfloat16)
    # permute patch-table elements: (i,j,c) -> col i*128 + pperm[j,c]
    eperm = np.empty(4 * C, np.int64)
    for i in range(2):
        for j in range(2):
            eperm[i * 128 + pperm0[j]] = (i * 2 + j) * C + cc
    xd_perm = xd_all[:, :, eperm]

    gy = np.clip(y0a, 0, H - 2)
    gx = np.clip(x0a, 0, W - 2)
    pidx = (gy * W + gx).astype(np.int32)
    # wcell remap for the gather path
    wcell = np.zeros((H, W, K, 2, 2), np.float32)
    for a_ in range(2):
        for b_ in range(2):
            for i in range(2):
                for j in range(2):
                    m = ((y0a + a_) == (gy + i)) & ((x0a + b_) == (gx + j))
                    wcell[..., i, j] += wts[..., a_, b_] * m

    totw = sum(ch["wfree"] for ch in plan)
    totn = sum(ch["nwpad"] for ch in plan)
    cols = np.arange(W)
    # partition permutation: logical (j,c) -> p = (c//16)*32 + j*16 + c%16
    cc = np.arange(C)
    pperm = np.empty((2, C), np.int64)
    for j in range(2):
        pperm[j] = (cc // 16) * 32 + j * 16 + (cc % 16)

    in_maps = []
    for core in range(NCORES):
        b, cm = core // 4, core % 4
        slab_c = np.empty((NCH, 128, AW * SLABW), bfloat16)  # paired later
        wc_c = np.zeros((2, 8, totw), np.float32)
        idx_c = np.zeros(max(totn, 16 * 16), np.int16)
        woff = ioff = 0
        for ch in plan:
            ci = ch["ci"]
            h = 4 * ci + cm
            # slab: [pperm[j,c], a*SLABW+col] = xp[b, h+a, col+j, c]
            for j in range(2):
                blk = xp[b, h:h + AW, j:SLABW + j, :]  # [AW, SLABW, C]
                slab_c[ci, pperm[j], :] = (
                    blk.transpose(2, 0, 1).reshape(C, AW * SLABW))
            # windowed weights
            for t in ch["win"]:
                k, a0, aw_, b0, bw_ = (t["k"], t["amin"], t["awid"],
                                       t["bmin"], t["bwid"])
                wjt = np.zeros((2, aw_, bw_ - 1, W), np.float32)
                ys_ = y0a[h, :, k] - h + 2 - a0
                bs_ = x0a[h, :, k] - cols - b0
                for i in range(2):
                    a = ys_ + i
                    ok = (a >= 0) & (a < aw_) & (bs_ >= 0) & (bs_ <= bw_ - 2)
                    idx = np.where(ok)[0]
                    for jj in range(2):
                        wjt[jj, a[idx], bs_[idx], idx] = wts[h, idx, k, i, jj]
                wc_c[:, :, woff + t["soff"] * W:
                     woff + (t["soff"] + t["nslot"]) * W] = (
                    wjt.reshape(2, 1, t["nslot"] * W))
            # wrap weights + indices
            nsl = ch["nslots"]
            for t in ch["wrap"]:
                k, wlo, wd, noff = t["k"], t["wlo"], t["width"], t["noff"]
                cl = np.arange(wlo, wlo + wd)
                idx_c[ioff + noff:ioff + noff + wd] = pidx[h, cl, k]
                for i in range(2):
                    base = woff + nsl * W + i * ch["nwpad"]
                    for jj in range(2):
                        wc_c[jj, :, base + noff:base + noff + wd] = (
                            wcell[h, cl, k, i, jj])
            woff += ch["wfree"]
            ioff += ch["nwpad"]
        # wrap idx layout [128, totn/16]
        n16 = max(totn // 16, 16)
        iw = idx_c[:n16 * 16].reshape(n16, 16).T.astype(np.int16)
        pp = np.arange(128)
        wfull = wc_c[((pp % 32) // 16), 0, :]
        in_maps.append({
            "slab": slab_c.reshape(NCH // 2, 2, 128, AW * SLABW)
                          .transpose(0, 2, 1, 3)
                          .reshape(NCH // 2, 128, 2 * AW * SLABW).copy(),
            "wcomp": wfull.astype(bfloat16),
            "widx": np.tile(iw, (8, 1)),
            "xd": xd_perm[b],
            "kdup": kdup_p,
        })
    return in_maps


_CACHE = {}
LAST_EXEC_NS = None


def kernel(x, kernel, scale, offset_base):
    global LAST_EXEC_NS
    x = np.asarray(x, np.float32)
    kern = np.asarray(kernel, np.float32)
    scale = np.asarray(scale, np.float32)
    offset_base = np.asarray(offset_base, np.float32)

    y0a, x0a, wts = geometry(scale, offset_base)
    plan = make_plan(y0a, x0a, wts)
    sig = plan_sig(plan)
    if sig not in _CACHE:
        _CACHE[sig] = build_bass(plan)
    nc = _CACHE[sig]

    in_maps = pack_inputs(x, kern, plan, y0a, x0a, wts)

    import os, sys, types
    trace = bool(os.environ.get("CHCONV_TRACE"))
    if trace:
        try:
            import antenv.axon_hooks  # noqa: F401
        except ImportError:
            from trn_agent_boot.trn_boot import _ntff_profile_via_ctypes
            hook = _ntff_profile_via_ctypes("/opt/axon/libaxon_pjrt.so")
            mod = types.ModuleType("antenv.axon_hooks")
            mod.get_axon_ntff_profile_hook = lambda: hook
            sys.modules["antenv.axon_hooks"] = mod
    res = run_bass_kernel_spmd(nc, in_maps, core_ids=list(range(NCORES)),
                               trace=trace)
    LAST_EXEC_NS = res.exec_time_ns

    out = np.empty((B, H, W, F), np.float32)
    for core in range(NCORES):
        o = np.asarray(res.results[core]["out"], np.float32)
        b, cm = core // 4, core % 4
        rows = o.reshape(F, NCH, W)
        for ci in range(NCH):
            out[b, 4 * ci + cm] = rows[:, ci, :].T
    return out



# revision 5
# speedup vs baseline: 1.0562x; 1.0562x over previous
"""Trainium2 Bass kernel for nn_CHConv — windowed deformable 3x3 conv, v4.

Design notes (v4):
- Geometry (offsets/windows/weights) is image-independent. Shard by
  (cm = h%4, xh = x-half): core = cm*2+xh handles rows {4j+cm}, columns
  [xh*128, xh*128+128), for BOTH images. The two images share one weight
  read (halves the dominant weight DMA vs batch-sharding).
- The core's column slice of BOTH images stays resident in SBUF,
  j-duplicated: partition (j*64+c) holds x[img, row, col+j, c] for a
  [136 row x 146 col] padded window. One chunk = one row-group; windowed
  taps read strided windows straight from the resident tile.
- T-build: DVE tensor_tensor T[p, slot, (img,x)] = resident-window * W,
  weights read once per img (re-read, img-shared).
- Seam taps (equirect wrap, dx ~ +204..255) use gpsimd.dma_gather with
  1024B elements that carry BOTH images' 2x2xC patch for a pixel.
- Matmuls: k-major accumulation into one PSUM bank [128F, 256=(img,x)]
  per chunk; wrap matmuls land on column intervals. ACT copies PSUM out
  as bf16.
"""
import numpy as np
from contextlib import ExitStack

import concourse.bass as bass
import concourse.bacc as bacc
import concourse.mybir as mybir
import concourse.tile as tile
from concourse.bass_utils import run_bass_kernel_spmd
from ml_dtypes import bfloat16

H, W, K, C, F, B = 128, 256, 9, 64, 128, 2
NCH = 32            # chunks (row-groups) per core
XW = 128            # x columns per core
RROWS = 136         # resident rows: image rows [cm-4, cm+132)
RCOLS = 146         # resident cols: image cols [xh*128-8, xh*128+138)
HALO = 8
A_MAX = 10
NPIX = H * W
NCORES = 8
RSZ = RROWS * RCOLS  # per-image resident block (cols within a partition)

_BF16 = mybir.dt.bfloat16
_F32 = mybir.dt.float32
_I16 = mybir.dt.int16


# ---------------------------------------------------------------- host plan
def geometry(scale, offset_base):
    off = (offset_base.astype(np.float32) * scale.astype(np.float32)).reshape(
        H, W, K, 2)
    ti, tj = np.meshgrid(np.arange(3), np.arange(3), indexing="ij")
    ti = ti.reshape(-1).astype(np.float32)
    tj = tj.reshape(-1).astype(np.float32)
    ys = (np.arange(H, dtype=np.float32)[:, None, None] - 1.0 + ti[None, None]
          + off[..., 0])
    xs = (np.arange(W, dtype=np.float32)[None, :, None] - 1.0 + tj[None, None]
          + off[..., 1])
    y0 = np.floor(ys); x0 = np.floor(xs)
    fy = (ys - y0).astype(np.float32); fx = (xs - x0).astype(np.float32)
    y0 = y0.astype(np.int64); x0 = x0.astype(np.int64)

    def v(yi, xi):
        return (((yi >= 0) & (yi < H) & (xi >= 0) & (xi < W))
                .astype(np.float32))
    w = np.zeros((H, W, K, 2, 2), np.float32)
    w[..., 0, 0] = (1 - fy) * (1 - fx) * v(y0, x0)
    w[..., 0, 1] = (1 - fy) * fx * v(y0, x0 + 1)
    w[..., 1, 0] = fy * (1 - fx) * v(y0 + 1, x0)
    w[..., 1, 1] = fy * fx * v(y0 + 1, x0 + 1)
    return y0, x0, w


def make_plan(y0, x0, w):
    """Per-chunk meta shared by all 8 cores (union over 4 rows x 2 halves).

    win tap: dict(k, amin, awid, bmin, bwid, soff, nslot)
    wrap tap: dict(k, lo, wd, noff)  -- lo/wd in LOCAL half coords (union of
      the two halves' local live intervals).
    """
    live = w.sum(axis=(3, 4)) > 0
    dy = y0 - np.arange(H)[:, None, None]
    dx = x0 - np.arange(W)[None, :, None]
    plan = []
    for ci in range(NCH):
        rows = [4 * ci + r for r in range(4)]
        win, wrap = [], []
        for k in range(K):
            lv = np.stack([live[h, :, k] for h in rows])
            if not lv.any():
                continue
            dyr = np.stack([dy[h, :, k] for h in rows])
            dxr = np.stack([dx[h, :, k] for h in rows])
            dmn, dmx = int(dxr[lv].min()), int(dxr[lv].max())
            ymn, ymx = int(dyr[lv].min()), int(dyr[lv].max())
            awid, bwid = ymx - ymn + 2, dmx - dmn + 2
            amin, bmin = ymn, dmn
            # window fits: cols 8+bmin+bs+x in [0,146), data col +1 padded;
            # rows 4ci+4+amin+ai in [0,136)
            ok = (bmin >= -HALO and bmin + bwid <= HALO + 3
                  and awid <= A_MAX
                  and 4 * ci + 4 + amin >= 0
                  and 4 * ci + 4 + amin + awid - 1 + 1 <= RROWS - 1)
            if ok:
                win.append(dict(k=k, amin=amin, awid=awid, bmin=bmin,
                                bwid=bwid, nslot=awid * (bwid - 1)))
            else:
                ws = np.where(lv.any(axis=0))[0]
                glo, ghi = int(ws.min()), int(ws.max())
                lo, hi = 127, 0
                for xh in range(2):
                    l_ = max(glo - 128 * xh, 0)
                    h_ = min(ghi - 128 * xh, 127)
                    if l_ <= h_:
                        lo = min(lo, l_); hi = max(hi, h_)
                assert lo <= hi
                wrap.append(dict(k=k, lo=lo, wd=hi - lo + 1))
        assert win, f"chunk {ci} has no windowed tap"
        soff = 0
        for t in win:
            t["soff"] = soff
            soff += t["nslot"]
        nw = sum(t["wd"] for t in wrap)
        nwpad = ((nw + 127) // 128) * 128 if nw else 0
        noff = 0
        for t in wrap:
            t["noff"] = noff
            noff += t["wd"]
        plan.append(dict(ci=ci, win=win, wrap=wrap, nslots=soff, nw=nw,
                         nwpad=nwpad, wfree=soff * XW + 2 * nwpad))
    return plan


def plan_sig(plan):
    sig = []
    for ch in plan:
        sig.append((ch["nslots"], ch["nwpad"],
                    tuple((t["k"], t["amin"], t["awid"], t["bmin"], t["bwid"])
                          for t in ch["win"]),
                    tuple((t["k"], t["lo"], t["wd"]) for t in ch["wrap"])))
    return tuple(sig)


# ---------------------------------------------------------------- bass build
def build_bass(plan):
    totw = sum(ch["wfree"] for ch in plan)
    totn = sum(ch["nwpad"] for ch in plan)
    nc = bacc.Bacc("TRN2", target_bir_lowering=False, debug=False)
    resid = nc.dram_tensor("resid", [2, 2, C, RSZ], _BF16,
                           kind="ExternalInput")          # [img, j, ch, r*c]
    wcomp = nc.dram_tensor("wcomp", [128, totw], _BF16, kind="ExternalInput")
    widx = nc.dram_tensor("widx", [128, max(totn // 16, 16)], _I16,
                          kind="ExternalInput")
    xdp = nc.dram_tensor("xdp", [NPIX, 8 * C], _BF16, kind="ExternalInput")
    kdup = nc.dram_tensor("kdup", [128, K * F], _BF16, kind="ExternalInput")
    out = nc.dram_tensor("out", [F, NCH * 256], _BF16, kind="ExternalOutput")

    woffs, ioffs = [], []
    wo = io = 0
    for ch in plan:
        woffs.append(wo); ioffs.append(io)
        wo += ch["wfree"]; io += ch["nwpad"]

    with ExitStack() as ctx:
        tc = ctx.enter_context(tile.TileContext(nc))
        rp = ctx.enter_context(tc.tile_pool(name="rp", bufs=1))
        kp = ctx.enter_context(tc.tile_pool(name="kp", bufs=1))
        idxp = ctx.enter_context(tc.tile_pool(name="idxp", bufs=1))
        wbp = ctx.enter_context(tc.tile_pool(name="wbp", bufs=3))
        twp = ctx.enter_context(tc.tile_pool(name="twp", bufs=2))
        gtp = ctx.enter_context(tc.tile_pool(name="gtp", bufs=3))
        trp = ctx.enter_context(tc.tile_pool(name="trp", bufs=2))
        op_ = ctx.enter_context(tc.tile_pool(name="op", bufs=3))
        psp = ctx.enter_context(tc.tile_pool(name="psp", bufs=4, space="PSUM"))

        res = rp.tile([128, 2 * RSZ], _BF16)
        # 4 DMAs: (img, j) -> partitions [j*64:(j+1)*64], cols img*RSZ
        for m in range(2):
            for j in range(2):
                nc.sync.dma_start(
                    out=res[j * 64:(j + 1) * 64, m * RSZ:(m + 1) * RSZ],
                    in_=resid[m, j, :, :])
        kd = kp.tile([128, K * F], _BF16)
        nc.sync.dma_start(out=kd[:], in_=kdup[:, :])
        idx_t = None
        if totn:
            idx_t = idxp.tile([128, totn // 16], _I16)
            nc.sync.dma_start(out=idx_t[:], in_=widx[:, 0:totn // 16])

        wbs, gts = {}, {}

        def stage_load(t):
            ch = plan[t]
            wbs[t] = wbp.tile([128, ch["wfree"]], _BF16, name=f"wb{t}",
                              tag="wb")
            nc.scalar.dma_start(out=wbs[t][:],
                                in_=wcomp[:, woffs[t]:woffs[t] + ch["wfree"]])

        def stage_gather(t):
            ch = plan[t]
            ng = ch["nwpad"]
            if ng:
                gts[t] = gtp.tile([128, 4, ng], _BF16, name=f"gt{t}",
                                  tag="gt")
                nc.gpsimd.dma_gather(
                    out_ap=gts[t][:], in_ap=xdp[:, :],
                    idxs_ap=idx_t[:, ioffs[t] // 16:(ioffs[t] + ng) // 16],
                    num_idxs=ng, num_idxs_reg=ng,
                    elem_size=8 * C, transpose=True, single_packet=False,
                )

        def stage_compute(t):
            ch = plan[t]
            wb = wbs.pop(t)
            ng = ch["nwpad"]
            nslots = ch["nslots"]
            tw = twp.tile([128, nslots, 256], _BF16, name=f"tw{t}", tag="tw")
            tr = (trp.tile([128, 2, ng], _BF16, name=f"tr{t}", tag="tr")
                  if ng else None)
            ps = psp.tile([128, 256], _F32, space="PSUM", name=f"ps{t}",
                          tag="ps")

            rbase = res[:]
            for tp in ch["win"]:
                a0, b0 = tp["amin"], tp["bmin"]
                aw_, bw_ = tp["awid"], tp["bwid"]
                s0 = tp["soff"]
                wsl = bass.AP(
                    wb.tensor,
                    wb.offset + tp["soff"] * XW,
                    [[wb.ap[0][0], 128], [XW * (bw_ - 1), aw_],
                     [XW, bw_ - 1], [1, XW]],
                )
                for m in range(2):
                    src4 = bass.AP(
                        rbase.tensor,
                        rbase.offset + m * RSZ
                        + (4 * t + 4 + a0) * RCOLS + HALO + b0,
                        [[rbase.ap[0][0], 128], [RCOLS, aw_], [1, bw_ - 1],
                         [1, XW]],
                    )
                    dst = bass.AP(
                        tw.tensor,
                        tw.offset + s0 * 256 + m * XW,
                        [[tw.ap[0][0], 128], [256 * (bw_ - 1), aw_],
                         [256, bw_ - 1], [1, XW]],
                    )
                    nc.vector.tensor_tensor(out=dst, in0=src4, in1=wsl,
                                            op=mybir.AluOpType.mult)
            if ng:
                gt = gts.pop(t)
                # gt: [128, (img,i), ng]; weights [128, (i), ng] img-shared
                w4 = bass.AP(
                    wb.tensor,
                    wb.offset + nslots * XW,
                    [[wb.ap[0][0], 128], [0, 2], [ng, 2], [1, ng]],
                )
                t4 = trp.tile([128, 4, ng], _BF16, name=f"t4{t}", tag="t4")
                nc.vector.tensor_tensor(out=t4[:], in0=gt[:], in1=w4,
                                        op=mybir.AluOpType.mult)
                # reduce i: tr[p, img, ng] = t4[p, img, 0, :] + t4[p, img, 1, :]
                t4v = t4[:].rearrange("p (m i) n -> p m i n", m=2)
                nc.vector.tensor_tensor(
                    out=tr[:], in0=t4v[:, :, 0, :], in1=t4v[:, :, 1, :],
                    op=mybir.AluOpType.add)

            # matmuls, k-major
            mm_list = []
            for tp in ch["win"]:
                for sx in range(tp["nslot"]):
                    mm_list.append((tp["k"], "w", tp["soff"] + sx))
            for tp in ch["wrap"]:
                for m in range(2):
                    mm_list.append((tp["k"], "g", (tp["lo"], tp["wd"],
                                                   tp["noff"], m)))
            mm_list.sort(key=lambda e: (0 if e[1] == "w" else 1, e[0]))
            # ensure first is full-width win (start=True clears all cols)
            last = len(mm_list) - 1
            for i, (k, kind, args) in enumerate(mm_list):
                st = (i == 0)
                stop = (i == last)
                if kind == "w":
                    nc.tensor.matmul(
                        ps[:], lhsT=kd[:, k * F:(k + 1) * F],
                        rhs=tw[:, args, :], start=st, stop=stop)
                else:
                    lo, wd, noff, m = args
                    nc.tensor.matmul(
                        ps[:, m * XW + lo:m * XW + lo + wd],
                        lhsT=kd[:, k * F:(k + 1) * F],
                        rhs=tr[:, m, noff:noff + wd],
                        start=False, stop=stop)

            ob = op_.tile([128, 256], _BF16, name=f"ob{t}", tag="ob")
            nc.scalar.copy(out=ob[:], in_=ps[:])
            nc.scalar.dma_start(out=out[:, t * 256:(t + 1) * 256], in_=ob[:])

        stage_load(0)
        stage_gather(0)
        for t in range(NCH + 2):
            if t + 1 < NCH:
                stage_load(t + 1)
                stage_gather(t + 1)
            if 0 <= t < NCH:
                stage_compute(t)
    nc.finalize()
    return nc


# ---------------------------------------------------------------- host pack
def pack_inputs(x, kern, plan, y0a, x0a, wts):
    xbf = x.astype(np.float32)
    # padded image: rows [-4, 136), cols [-8, 267)
    xp = np.zeros((B, 140, 275, C), np.float32)
    xp[:, 4:4 + H, 8:8 + W, :] = xbf
    xp = xp.astype(bfloat16)

    # patch table with both images packed per pixel: [pix, img, i, j, c]
    xpad = np.pad(xbf, [(0, 0), (0, 1), (0, 1), (0, 0)])
    xd = np.empty((H, W, 2, 2, 2, C), np.float32)
    for i in range(2):
        for j in range(2):
            xd[:, :, :, i, j, :] = xpad[:, i:i + H, j:j + W, :].transpose(
                1, 2, 0, 3)
    xdp = xd.reshape(NPIX, 8 * C).astype(bfloat16)

    km = kern.reshape(F, C, K).astype(np.float32)
    kdv = km.transpose(1, 2, 0).reshape(C, K * F)
    kdup = np.concatenate([kdv, kdv], axis=0).astype(bfloat16)

    totw = sum(ch["wfree"] for ch in plan)
    totn = sum(ch["nwpad"] for ch in plan)

    gy = np.clip(y0a, 0, H - 2)
    gx = np.clip(x0a, 0, W - 2)
    pidx = (gy * W + gx).astype(np.int32)
    # effective 2x2-cell weights after clipping shift
    wcell = np.zeros((H, W, K, 2, 2), np.float32)
    for a_ in range(2):
        for b_ in range(2):
            for i in range(2):
                for j in range(2):
                    m = ((y0a + a_) == (gy + i)) & ((x0a + b_) == (gx + j))
                    wcell[..., i, j] += wts[..., a_, b_] * m

    in_maps = []
    for core in range(NCORES):
        cm, xh = core // 2, core % 2
        # resident: [img, j, RROWS*RCOLS, C]
        resid = np.empty((2, 2, C, RROWS, RCOLS), bfloat16)
        for j in range(2):
            resid[:, j] = xp[:, cm:cm + RROWS,
                             xh * 128 + j:xh * 128 + j + RCOLS, :].transpose(
                                 0, 3, 1, 2)
        resid = resid.reshape(2, 2, C, RSZ)

        wc = np.zeros((2, totw), np.float32)   # [j, totw]
        idx_c = np.zeros(max(totn, 256), np.int16)
        xls = np.arange(XW)
        gxs = xh * 128 + xls
        wo = io = 0
        for ch in plan:
            ci = ch["ci"]
            h = 4 * ci + cm
            for t in ch["win"]:
                k, a0, aw_, b0, bw_ = (t["k"], t["amin"], t["awid"],
                                       t["bmin"], t["bwid"])
                wjt = np.zeros((2, aw_, bw_ - 1, XW), np.float32)
                dyv = y0a[h, gxs, k] - h - a0
                bsv = x0a[h, gxs, k] - gxs - b0
                for i in range(2):
                    a = dyv + i
                    ok = (a >= 0) & (a < aw_) & (bsv >= 0) & (bsv <= bw_ - 2)
                    ii = np.where(ok)[0]
                    for jj in range(2):
                        wjt[jj, a[ii], bsv[ii], ii] = wts[h, gxs[ii], k, i, jj]
                wc[:, wo + t["soff"] * XW:
                   wo + (t["soff"] + t["nslot"]) * XW] = (
                    wjt.reshape(2, t["nslot"] * XW))
            nsl = ch["nslots"]
            for t in ch["wrap"]:
                k, lo, wd, noff = t["k"], t["lo"], t["wd"], t["noff"]
                gcl = xh * 128 + np.arange(lo, lo + wd)
                valid = gcl < W
                gc = np.clip(gcl, 0, W - 1)
                idx_c[io + noff:io + noff + wd] = np.where(
                    valid, pidx[h, gc, k], 0)
                for i in range(2):
                    base = wo + nsl * XW + i * ch["nwpad"]
                    for jj in range(2):
                        wv = np.where(valid, wcell[h, gc, k, i, jj], 0.0)
                        wc[jj, base + noff:base + noff + wd] = wv
            wo += ch["wfree"]
            io += ch["nwpad"]

        pp = np.arange(128)
        wfull = wc[pp // 64, :].astype(bfloat16)
        n16 = max(totn // 16, 16)
        iw = idx_c[:n16 * 16].reshape(n16, 16).T.astype(np.int16)
        in_maps.append({
            "resid": resid,
            "wcomp": wfull,
            "widx": np.tile(iw, (8, 1)),
            "xdp": xdp,
            "kdup": kdup,
        })
    return in_maps


_CACHE = {}
LAST_EXEC_NS = None


def kernel(x, kernel, scale, offset_base):
    global LAST_EXEC_NS
    x = np.asarray(x, np.float32)
    kern = np.asarray(kernel, np.float32)
    scale = np.asarray(scale, np.float32)
    offset_base = np.asarray(offset_base, np.float32)

    y0a, x0a, wts = geometry(scale, offset_base)
    plan = make_plan(y0a, x0a, wts)
    sig = plan_sig(plan)
    if sig not in _CACHE:
        _CACHE[sig] = build_bass(plan)
    nc = _CACHE[sig]

    in_maps = pack_inputs(x, kern, plan, y0a, x0a, wts)

    import os, sys, types
    trace = bool(os.environ.get("CHCONV_TRACE"))
    if trace:
        try:
            import antenv.axon_hooks  # noqa: F401
        except ImportError:
            from trn_agent_boot.trn_boot import _ntff_profile_via_ctypes
            hook = _ntff_profile_via_ctypes("/opt/axon/libaxon_pjrt.so")
            mod = types.ModuleType("antenv.axon_hooks")
            mod.get_axon_ntff_profile_hook = lambda: hook
            sys.modules["antenv.axon_hooks"] = mod
    res = run_bass_kernel_spmd(nc, in_maps, core_ids=list(range(NCORES)),
                               trace=trace)
    LAST_EXEC_NS = res.exec_time_ns

    out = np.empty((B, H, W, F), np.float32)
    for core in range(NCORES):
        cm, xh = core // 2, core % 2
        o = np.asarray(res.results[core]["out"], np.float32)
        o = o.reshape(F, NCH, 2, XW)        # [f, ci, img, xl]
        for m in range(B):
            out[m, cm::4, xh * 128:xh * 128 + XW, :] = (
                o[:, :, m, :].transpose(1, 2, 0))
    return out


# revision 12
# speedup vs baseline: 1.4098x; 1.3348x over previous
"""Trainium2 Bass kernel for nn_CHConv — windowed deformable 3x3 conv, v5.

Design notes:
- Geometry (offsets/windows/weights) is image-independent. Shard by
  (cm = h%4, xh = x-half): core = cm*2+xh handles rows {4j+cm}, columns
  [xh*128, xh*128+128), for BOTH images. The two images share one weight
  read (halves the dominant weight DMA vs batch-sharding).
- The core's column slice of BOTH images stays resident in SBUF,
  j-duplicated: partition (j*64+c) holds x[img, row, col+j, c] for a
  [136 row x 146 col] padded window. One chunk = one row-group; windowed
  taps read strided windows straight from the resident tile.
- T-build: one 5-dim DVE tensor_tensor per tap covers both images
  (weights read once, img dim stride-0 on the weight AP).
- Seam taps (equirect wrap, dx ~ +204..255) use gpsimd.dma_gather with
  1024B elements carrying BOTH images' 2x2xC patch; gathers are grouped
  8 chunks per call (padding only per group) to cut SWDGE descriptor
  generation cost.
- Matmuls: k-major accumulation into one PSUM bank [128F, 256=(img,x)]
  per chunk; wrap matmuls land on column intervals. ACT copies PSUM out
  as bf16.
"""
import numpy as np
from contextlib import ExitStack

import concourse.bass as bass
import concourse.bacc as bacc
import concourse.mybir as mybir
import concourse.tile as tile
from concourse.bass_utils import run_bass_kernel_spmd
from ml_dtypes import bfloat16

H, W, K, C, F, B = 128, 256, 9, 64, 128, 2
NCH = 32            # chunks (row-groups) per core
GSZ = 8             # chunks per gather group
NG = NCH // GSZ     # gather groups
XW = 128            # x columns per core
RROWS = 136         # resident rows: image rows [cm-4, cm+132)
RCOLS = 146         # resident cols: image cols [xh*128-8, xh*128+138)
HALO = 8
A_MAX = 10
NPIX = H * W
NCORES = 8
RSZ = RROWS * RCOLS

_BF16 = mybir.dt.bfloat16
_F32 = mybir.dt.float32
_I16 = mybir.dt.int16


# ---------------------------------------------------------------- host plan
def geometry(scale, offset_base):
    off = (offset_base.astype(np.float32) * scale.astype(np.float32)).reshape(
        H, W, K, 2)
    ti, tj = np.meshgrid(np.arange(3), np.arange(3), indexing="ij")
    ti = ti.reshape(-1).astype(np.float32)
    tj = tj.reshape(-1).astype(np.float32)
    ys = (np.arange(H, dtype=np.float32)[:, None, None] - 1.0 + ti[None, None]
          + off[..., 0])
    xs = (np.arange(W, dtype=np.float32)[None, :, None] - 1.0 + tj[None, None]
          + off[..., 1])
    y0 = np.floor(ys); x0 = np.floor(xs)
    fy = (ys - y0).astype(np.float32); fx = (xs - x0).astype(np.float32)
    y0 = y0.astype(np.int64); x0 = x0.astype(np.int64)

    def v(yi, xi):
        return (((yi >= 0) & (yi < H) & (xi >= 0) & (xi < W))
                .astype(np.float32))
    w = np.zeros((H, W, K, 2, 2), np.float32)
    w[..., 0, 0] = (1 - fy) * (1 - fx) * v(y0, x0)
    w[..., 0, 1] = (1 - fy) * fx * v(y0, x0 + 1)
    w[..., 1, 0] = fy * (1 - fx) * v(y0 + 1, x0)
    w[..., 1, 1] = fy * fx * v(y0 + 1, x0 + 1)
    return y0, x0, w


def make_plan(y0, x0, w):
    """Per-chunk meta shared by all 8 cores (union over 4 rows x 2 halves)."""
    live = w.sum(axis=(3, 4)) > 0
    dy = y0 - np.arange(H)[:, None, None]
    dx = x0 - np.arange(W)[None, :, None]
    plan = []
    for ci in range(NCH):
        rows = [4 * ci + r for r in range(4)]
        win, wrap = [], []
        for k in range(K):
            lv = np.stack([live[h, :, k] for h in rows])
            if not lv.any():
                continue
            dyr = np.stack([dy[h, :, k] for h in rows])
            dxr = np.stack([dx[h, :, k] for h in rows])
            dmn, dmx = int(dxr[lv].min()), int(dxr[lv].max())
            ymn, ymx = int(dyr[lv].min()), int(dyr[lv].max())
            awid, bwid = ymx - ymn + 2, dmx - dmn + 2
            amin, bmin = ymn, dmn
            ok = (bmin >= -HALO and bmin + bwid <= HALO + 3
                  and awid <= A_MAX
                  and 4 * ci + 4 + amin >= 0
                  and 4 * ci + 4 + amin + awid <= RROWS)
            if ok:
                win.append(dict(k=k, amin=amin, awid=awid, bmin=bmin,
                                bwid=bwid, nslot=awid * (bwid - 1)))
            else:
                ws = np.where(lv.any(axis=0))[0]
                glo, ghi = int(ws.min()), int(ws.max())
                lo, hi = 127, 0
                for xh in range(2):
                    l_ = max(glo - 128 * xh, 0)
                    h_ = min(ghi - 128 * xh, 127)
                    if l_ <= h_:
                        lo = min(lo, l_); hi = max(hi, h_)
                assert lo <= hi
                wrap.append(dict(k=k, lo=lo, wd=hi - lo + 1))
        assert win, f"chunk {ci} has no windowed tap"
        soff = 0
        for t in win:
            t["soff"] = soff
            soff += t["nslot"]
        nw = sum(t["wd"] for t in wrap)
        noff = 0
        for t in wrap:
            t["noff"] = noff
            noff += t["wd"]
        plan.append(dict(ci=ci, win=win, wrap=wrap, nslots=soff, nw=nw,
                         wfree=soff * XW))
    # gather groups: adaptive — big-gather chunks go solo, interior chunks
    # pack greedily up to ~1200 idxs. Chunk raw offsets within group; group
    # ng padded to 128.
    groups = []
    cur, cur_nw = [], 0
    for ci, ch in enumerate(plan):
        big = ch["nw"] > 400
        if cur and (big or cur_nw + ch["nw"] > 1200):
            groups.append(cur)
            cur, cur_nw = [], 0
        cur.append(ci)
        cur_nw += ch["nw"]
        if big:
            groups.append(cur)
            cur, cur_nw = [], 0
    if cur:
        groups.append(cur)
    gmeta = []
    for g, chs in enumerate(groups):
        off = 0
        for ci in chs:
            plan[ci]["goff"] = off
            plan[ci]["grp"] = g
            off += plan[ci]["nw"]
        ngg = ((off + 127) // 128) * 128 if off else 0
        gmeta.append(dict(g=g, ngg=ngg, raw=off, chunks=tuple(chs)))
    return plan, gmeta


def plan_sig(plan, groups):
    sig = []
    for ch in plan:
        sig.append((ch["nslots"], ch["nw"], ch["goff"], ch["grp"],
                    tuple((t["k"], t["amin"], t["awid"], t["bmin"], t["bwid"])
                          for t in ch["win"]),
                    tuple((t["k"], t["lo"], t["wd"]) for t in ch["wrap"])))
    sig.append(tuple((gr["ngg"], gr["chunks"]) for gr in groups))
    return tuple(sig)


# ---------------------------------------------------------------- bass build
def build_bass(plan, groups):
    totw = sum(ch["wfree"] for ch in plan)
    totng = sum(gr["ngg"] for gr in groups)
    goffs = np.cumsum([0] + [gr["ngg"] for gr in groups])[:-1]

    nc = bacc.Bacc("TRN2", target_bir_lowering=False, debug=False)
    resid = nc.dram_tensor("resid", [2, 128, RSZ], _BF16,
                           kind="ExternalInput")          # [img, (j,c), r*c]
    wcomp = nc.dram_tensor("wcomp", [128, totw], _BF16, kind="ExternalInput")
    wgr = nc.dram_tensor("wgr", [128, 2 * totng], _BF16,
                         kind="ExternalInput")            # per-group [i2, ngg]
    widx = nc.dram_tensor("widx", [128, max(totng // 16, 16)], _I16,
                          kind="ExternalInput")
    xdp = nc.dram_tensor("xdp", [NPIX, 8 * C], _BF16, kind="ExternalInput")
    kdup = nc.dram_tensor("kdup", [128, K * F], _BF16, kind="ExternalInput")
    out = nc.dram_tensor("out", [F, NCH * 256], _BF16, kind="ExternalOutput")

    woffs = np.cumsum([0] + [ch["wfree"] for ch in plan])[:-1]

    with ExitStack() as ctx:
        tc = ctx.enter_context(tile.TileContext(nc))
        rp = ctx.enter_context(tc.tile_pool(name="rp", bufs=1))
        kp = ctx.enter_context(tc.tile_pool(name="kp", bufs=1))
        idxp = ctx.enter_context(tc.tile_pool(name="idxp", bufs=1))
        wbp = ctx.enter_context(tc.tile_pool(name="wbp", bufs=3))
        wgp = ctx.enter_context(tc.tile_pool(name="wgp", bufs=2))
        twp = ctx.enter_context(tc.tile_pool(name="twp", bufs=2))
        gtp = ctx.enter_context(tc.tile_pool(name="gtp", bufs=2))
        trp = ctx.enter_context(tc.tile_pool(name="trp", bufs=2))
        op_ = ctx.enter_context(tc.tile_pool(name="op", bufs=3))
        psp = ctx.enter_context(tc.tile_pool(name="psp", bufs=4, space="PSUM"))

        res = rp.tile([128, 2 * RSZ], _BF16)
        for m in range(2):
            nc.sync.dma_start(out=res[:, m * RSZ:(m + 1) * RSZ],
                              in_=resid[m, :, :])
        kd = kp.tile([128, K * F], _BF16)
        nc.sync.dma_start(out=kd[:], in_=kdup[:, :])
        idx_t = idxp.tile([128, totng // 16], _I16)
        nc.sync.dma_start(out=idx_t[:], in_=widx[:, 0:totng // 16])

        wbs, gts, wgs, trs = {}, {}, {}, {}

        def stage_load(t):
            ch = plan[t]
            wbs[t] = wbp.tile([128, ch["wfree"]], _BF16, name=f"wb{t}",
                              tag="wb")
            nc.scalar.dma_start(out=wbs[t][:],
                                in_=wcomp[:, woffs[t]:woffs[t] + ch["wfree"]])

        def stage_gather(g):
            ngg = groups[g]["ngg"]
            if not ngg:
                return
            gts[g] = gtp.tile([128, 4, ngg], _BF16, name=f"gt{g}", tag="gt")
            wgs[g] = wgp.tile([128, 2, ngg], _BF16, name=f"wg{g}", tag="wg")
            go = int(goffs[g])
            nc.sync.dma_start(out=wgs[g][:],
                              in_=wgr[:, 2 * go:2 * (go + ngg)])
            nc.gpsimd.dma_gather(
                out_ap=gts[g][:], in_ap=xdp[:, :],
                idxs_ap=idx_t[:, go // 16:(go + ngg) // 16],
                num_idxs=ngg, num_idxs_reg=ngg,
                elem_size=8 * C, transpose=True, single_packet=False,
            )

        def stage_wrap_weight(g):
            ngg = groups[g]["ngg"]
            if not ngg:
                return
            gt, wg = gts.pop(g), wgs.pop(g)
            t4 = trp.tile([128, 4, ngg], _BF16, name=f"t4{g}", tag="t4")
            w4 = bass.AP(wg.tensor, wg.offset,
                         [[wg.ap[0][0], 128], [0, 2], [ngg, 2], [1, ngg]])
            nc.vector.tensor_tensor(out=t4[:], in0=gt[:], in1=w4,
                                    op=mybir.AluOpType.mult)
            tr = trp.tile([128, 2, ngg], _BF16, name=f"tr{g}", tag="tr")
            t4v = t4[:].rearrange("p (m i) n -> p m i n", m=2)
            nc.vector.tensor_tensor(
                out=tr[:], in0=t4v[:, :, 0, :], in1=t4v[:, :, 1, :],
                op=mybir.AluOpType.add)
            trs[g] = tr

        def stage_compute(t):
            ch = plan[t]
            g = ch["grp"]
            wb = wbs.pop(t)
            nslots = ch["nslots"]
            tw = twp.tile([128, nslots, 256], _BF16, name=f"tw{t}", tag="tw")
            ps = psp.tile([128, 256], _F32, space="PSUM", name=f"ps{t}",
                          tag="ps")

            rbase = res[:]
            for tp in ch["win"]:
                a0, b0 = tp["amin"], tp["bmin"]
                aw_, bw_ = tp["awid"], tp["bwid"]
                s0 = tp["soff"]
                wsl = bass.AP(
                    wb.tensor,
                    wb.offset + s0 * XW,
                    [[wb.ap[0][0], 128], [XW * (bw_ - 1), aw_],
                     [XW, bw_ - 1], [1, XW]],
                )
                for m in range(2):
                    src4 = bass.AP(
                        rbase.tensor,
                        rbase.offset + m * RSZ
                        + (4 * t + 4 + a0) * RCOLS + HALO + b0,
                        [[rbase.ap[0][0], 128], [RCOLS, aw_], [1, bw_ - 1],
                         [1, XW]],
                    )
                    dst4 = bass.AP(
                        tw.tensor,
                        tw.offset + s0 * 256 + m * XW,
                        [[tw.ap[0][0], 128], [256 * (bw_ - 1), aw_],
                         [256, bw_ - 1], [1, XW]],
                    )
                    nc.vector.tensor_tensor(out=dst4, in0=src4, in1=wsl,
                                            op=mybir.AluOpType.mult)

            tr = trs.get(g)
            goff_c = ch["goff"]
            mm_list = []
            for tp in ch["win"]:
                for sx in range(tp["nslot"]):
                    mm_list.append((tp["k"], "w", tp["soff"] + sx))
            for tp in ch["wrap"]:
                for m in range(2):
                    mm_list.append((tp["k"], "g", (tp["lo"], tp["wd"],
                                                   tp["noff"], m)))
            mm_list.sort(key=lambda e: (0 if e[1] == "w" else 1, e[0]))
            last = len(mm_list) - 1
            for i, (k, kind, args) in enumerate(mm_list):
                st = (i == 0)
                stop = (i == last)
                if kind == "w":
                    nc.tensor.matmul(
                        ps[:], lhsT=kd[:, k * F:(k + 1) * F],
                        rhs=tw[:, args, :], start=st, stop=stop)
                else:
                    lo, wd, noff, m = args
                    nc.tensor.matmul(
                        ps[:, m * XW + lo:m * XW + lo + wd],
                        lhsT=kd[:, k * F:(k + 1) * F],
                        rhs=tr[:, m, goff_c + noff:goff_c + noff + wd],
                        start=False, stop=stop)

            ob = op_.tile([128, 256], _BF16, name=f"ob{t}", tag="ob")
            nc.scalar.copy(out=ob[:], in_=ps[:])
            nc.scalar.dma_start(out=out[:, t * 256:(t + 1) * 256], in_=ob[:])

        stage_load(0)
        stage_load(1)
        stage_gather(0)
        if len(groups) > 1:
            stage_gather(1)
        for t in range(NCH):
            g = plan[t]["grp"]
            if t == groups[g]["chunks"][0]:
                stage_wrap_weight(g)
                if g + 2 < len(groups):
                    stage_gather(g + 2)
            if t + 2 < NCH:
                stage_load(t + 2)
            stage_compute(t)
    nc.finalize()
    return nc


# ---------------------------------------------------------------- host pack
def pack_inputs(x, kern, plan, groups, y0a, x0a, wts):
    xbf = x.astype(np.float32)
    xp = np.zeros((B, 140, 275, C), np.float32)
    xp[:, 4:4 + H, 8:8 + W, :] = xbf
    xp = xp.astype(bfloat16)

    xpad = np.pad(xbf, [(0, 0), (0, 1), (0, 1), (0, 0)])
    xd = np.empty((H, W, 2, 2, 2, C), np.float32)
    for i in range(2):
        for j in range(2):
            xd[:, :, :, i, j, :] = xpad[:, i:i + H, j:j + W, :].transpose(
                1, 2, 0, 3)
    xdp = xd.reshape(NPIX, 8 * C).astype(bfloat16)

    km = kern.reshape(F, C, K).astype(np.float32)
    kdv = km.transpose(1, 2, 0).reshape(C, K * F)
    kdup = np.concatenate([kdv, kdv], axis=0).astype(bfloat16)

    totw = sum(ch["wfree"] for ch in plan)
    totng = sum(gr["ngg"] for gr in groups)
    goffs = np.cumsum([0] + [gr["ngg"] for gr in groups])[:-1]

    gy = np.clip(y0a, 0, H - 2)
    gx = np.clip(x0a, 0, W - 2)
    pidx = (gy * W + gx).astype(np.int32)
    wcell = np.zeros((H, W, K, 2, 2), np.float32)
    for a_ in range(2):
        for b_ in range(2):
            for i in range(2):
                for j in range(2):
                    m = ((y0a + a_) == (gy + i)) & ((x0a + b_) == (gx + j))
                    wcell[..., i, j] += wts[..., a_, b_] * m

    in_maps = []
    for core in range(NCORES):
        cm, xh = core // 2, core % 2
        resid = np.empty((2, 2, C, RROWS, RCOLS), bfloat16)
        for j in range(2):
            resid[:, j] = xp[:, cm:cm + RROWS,
                             xh * 128 + j:xh * 128 + j + RCOLS, :].transpose(
                                 0, 3, 1, 2)
        resid = resid.reshape(2, 128, RSZ)

        wc = np.zeros((2, totw), np.float32)        # [j, totw] win weights
        wg = np.zeros((2, 2, totng), np.float32)    # [j, i, totng] wrap w
        idx_c = np.zeros(max(totng, 256), np.int16)
        xls = np.arange(XW)
        gxs = xh * 128 + xls
        for ch in plan:
            ci = ch["ci"]
            h = 4 * ci + cm
            wo = ch["wfree"] and 0
            wo = 0
            for t in ch["win"]:
                k, a0, aw_, b0, bw_ = (t["k"], t["amin"], t["awid"],
                                       t["bmin"], t["bwid"])
                wjt = np.zeros((2, aw_, bw_ - 1, XW), np.float32)
                dyv = y0a[h, gxs, k] - h - a0
                bsv = x0a[h, gxs, k] - gxs - b0
                for i in range(2):
                    a = dyv + i
                    ok = (a >= 0) & (a < aw_) & (bsv >= 0) & (bsv <= bw_ - 2)
                    ii = np.where(ok)[0]
                    for jj in range(2):
                        wjt[jj, a[ii], bsv[ii], ii] = wts[h, gxs[ii], k, i, jj]
                base = int(np.sum([c2["wfree"] for c2 in plan[:ci]]))
                wc[:, base + t["soff"] * XW:
                   base + (t["soff"] + t["nslot"]) * XW] = (
                    wjt.reshape(2, t["nslot"] * XW))
            go = int(goffs[ch["grp"]]) + ch["goff"]
            for t in ch["wrap"]:
                k, lo, wd, noff = t["k"], t["lo"], t["wd"], t["noff"]
                gcl = xh * 128 + np.arange(lo, lo + wd)
                valid = gcl < W
                gc = np.clip(gcl, 0, W - 1)
                idx_c[go + noff:go + noff + wd] = np.where(
                    valid, pidx[h, gc, k], 0)
                for i in range(2):
                    for jj in range(2):
                        wv = np.where(valid, wcell[h, gc, k, i, jj], 0.0)
                        wg[jj, i, go + noff:go + noff + wd] = wv
        pp = np.arange(128)
        wfull = wc[pp // 64, :].astype(bfloat16)
        # wgr layout per group: [128, (i2, ngg)] regions concatenated
        wgfull = np.zeros((128, 2 * totng), np.float32)
        for gr in groups:
            g, ngg = gr["g"], gr["ngg"]
            if not ngg:
                continue
            go = int(goffs[g])
            blk = wg[:, :, go:go + ngg]            # [j, i, ngg]
            wgfull[:, 2 * go:2 * (go + ngg)] = (
                blk[pp // 64].reshape(128, 2 * ngg))
        n16 = max(totng // 16, 16)
        iw = idx_c[:n16 * 16].reshape(n16, 16).T.astype(np.int16)
        in_maps.append({
            "resid": resid,
            "wcomp": wfull,
            "wgr": wgfull.astype(bfloat16),
            "widx": np.tile(iw, (8, 1)),
            "xdp": xdp,
            "kdup": kdup,
        })
    return in_maps


_CACHE = {}
LAST_EXEC_NS = None


def kernel(x, kernel, scale, offset_base):
    global LAST_EXEC_NS
    x = np.asarray(x, np.float32)
    kern = np.asarray(kernel, np.float32)
    scale = np.asarray(scale, np.float32)
    offset_base = np.asarray(offset_base, np.float32)

    y0a, x0a, wts = geometry(scale, offset_base)
    plan, groups = make_plan(y0a, x0a, wts)
    sig = plan_sig(plan, groups)
    if sig not in _CACHE:
        _CACHE[sig] = build_bass(plan, groups)
    nc = _CACHE[sig]

    in_maps = pack_inputs(x, kern, plan, groups, y0a, x0a, wts)

    import os, sys, types
    trace = bool(os.environ.get("CHCONV_TRACE"))
    if trace:
        try:
            import antenv.axon_hooks  # noqa: F401
        except ImportError:
            from trn_agent_boot.trn_boot import _ntff_profile_via_ctypes
            hook = _ntff_profile_via_ctypes("/opt/axon/libaxon_pjrt.so")
            mod = types.ModuleType("antenv.axon_hooks")
            mod.get_axon_ntff_profile_hook = lambda: hook
            sys.modules["antenv.axon_hooks"] = mod
    res = run_bass_kernel_spmd(nc, in_maps, core_ids=list(range(NCORES)),
                               trace=trace)
    LAST_EXEC_NS = res.exec_time_ns

    out = np.empty((B, H, W, F), np.float32)
    for core in range(NCORES):
        cm, xh = core // 2, core % 2
        o = np.asarray(res.results[core]["out"], np.float32)
        o = o.reshape(F, NCH, 2, XW)
        for m in range(B):
            out[m, cm::4, xh * 128:xh * 128 + XW, :] = (
                o[:, :, m, :].transpose(1, 2, 0))
    return out
